# revision 31
# baseline (speedup 1.0000x reference)
"""Distributed Bass kernel for nn_Interaction_GraphConvolution.

Math (reference):
    x  = node_features @ linear_w.T + linear_b          [N, IN_F]
    wf = x @ weight                                     [N, C]
    G  = mask_father[:,0,:].T @ adjacency               [N, N]
    P  = G * mask_hadamard[:,0,:].T                     [N, N]
    out[c, j] = wf[j,c] * (P @ wf)[j,c] / neighbor_count[c]^2

Host folds the two linear layers (FW = lw.T @ W, fb = lb @ W) and the
normalization (FW2 = FW / ncnt^2, fb2 = fb / ncnt^2).  The big GEMM is
factored through the linear layer:
    P @ wf = (P @ nf) @ FW + rowsum(P) x fb
which is 2*N^2*F_RAW + 2*N*F_RAW*C flops instead of 2*N^2*C (4x less).

Single SPMD launch, output columns j (node dim) split across 8 cores:
  phase G: G rows J_m via fp8e4 DoubleRow (adjacency/mask are 0/1 -
           exact), multiply by S^T, DMA-XBAR transpose into P^T.
  phase X: XT = nf^T @ P^T (bf16).
  phase O: out[j,c] = (XT.T@FW + rsum*fb) * (nfT.T@FW2 + fb2).
Matmuls keep one 128x128 stationary across multiple 512-wide moving
chunks so LDWEIGHTS amortizes.  Inputs are host-packed into SBUF layout
so every DMA moves >=4KB per partition line (descriptor-rate bound).
"""

import os
import sys

sys.path.insert(0, "/opt/trn_rl_repo")

import numpy as np
import ml_dtypes

from concourse import bass, bacc, mybir, tile
from concourse.bass_utils import run_bass_kernel_spmd

F32 = mybir.dt.float32
BF16 = mybir.dt.bfloat16
F8E4 = mybir.dt.float8e4
DR = mybir.MatmulPerfMode.DoubleRow

N = 4096       # nodes (== out channels C)
F_RAW = 512    # raw feature dim
IN_F = 1024    # hidden dim
C = 4096       # out channels
M = 8          # cores
JB = N // M    # 512 output columns per core

NKP = N // 256    # 16 k-pairs (DoubleRow contracts 256 per pass)
NKB = N // 128    # 32 k-blocks
NIB = N // 128    # 32 i-blocks
NJB = JB // 128   # 4 j-blocks
NIC = N // 512    # 8 i-chunks of 512
NRB = F_RAW // 128  # 4 r-blocks
CQ = 1024         # c quarter
NCQ = C // CQ     # 4

LAST_EXEC = {}
LAST_RESULTS = {}


def _build_neff():
    nc = bacc.Bacc()
    a_d = nc.dram_tensor("a8", [128, NKB, N], F8E4, kind="ExternalInput")
    ao_d = nc.dram_tensor("ao8", [128, NKB, JB], F8E4, kind="ExternalInput")
    st_d = nc.dram_tensor("stT", [JB, N], BF16, kind="ExternalInput")
    nf_d = nc.dram_tensor("nfb", [128, NIB, F_RAW], BF16, kind="ExternalInput")
    nfT_d = nc.dram_tensor("nfT", [128, NRB, JB], BF16, kind="ExternalInput")
    fw_d = nc.dram_tensor("fwt", [128, NRB, C], BF16, kind="ExternalInput")
    fw2_d = nc.dram_tensor("fw2", [128, NRB, C], BF16, kind="ExternalInput")
    fbt_d = nc.dram_tensor("fbt", [128, C], BF16, kind="ExternalInput")
    fb2_d = nc.dram_tensor("fb2", [128, C], BF16, kind="ExternalInput")
    out_d = nc.dram_tensor("outc", [JB, C], F32, kind="ExternalOutput")

    NLO = 12  # nf i-blocks prefetched during phase G (SBUF budget)

    with tile.TileContext(nc) as tc:
        with tc.tile_pool(name="const", bufs=1) as constp:
            # P^T: ptT[p, ib, j] = P[j, ib*128+p], i on partitions
            ptT_t = constp.tile([128, NIB, JB], BF16)
            rsum_t = constp.tile([128, NJB], F32)  # rowsum(P), col per j-block
            nflo_t = constp.tile([128, NLO, F_RAW], BF16)

            # ---- phase G: G rows J_m (fp8 DoubleRow), *S^T, transpose ----
            with tc.tile_pool(name="ga", bufs=1) as gap, \
                 tc.tile_pool(name="stp", bufs=1) as stp, \
                 tc.tile_pool(name="pgp", bufs=1) as pgp, \
                 tc.tile_pool(name="psG", bufs=8,
                              space=bass.MemorySpace.PSUM) as psgp:
                aot_t = gap.tile([128, NKP, 2, JB], F8E4)
                nc.sync.dma_start(aot_t[:], ao_d[:])
                st0_t = stp.tile([128, N], BF16, tag="st", name="st0")
                nc.sync.dma_start(st0_t[:], st_d[0:128, :])
                a_t = gap.tile([128, NKP, 2, N], F8E4)
                for kp in range(NKP):
                    nc.sync.dma_start(
                        a_t[:, kp, :, :], a_d[:, 2 * kp:2 * kp + 2, :])
                nc.sync.dma_start(nflo_t[:], nf_d[:, 0:NLO, :])

                for jb in range(NJB):
                    if jb == 0:
                        st_t = st0_t
                    else:
                        st_t = stp.tile([128, N], BF16, tag="st",
                                        name=f"st{jb}")
                        nc.sync.dma_start(
                            st_t[:], st_d[jb * 128:(jb + 1) * 128, :])
                    pg_sb = pgp.tile([128, N], BF16, tag="pg", name=f"pg{jb}")
                    psg = [psgp.tile([128, 512], F32, tag="g",
                                     name=f"g{jb}_{i}") for i in range(NIC)]
                    for kp in range(NKP):
                        lhs = aot_t[:, kp, :, jb * 128:(jb + 1) * 128]
                        for ic in range(NIC):
                            nc.tensor.matmul(
                                psg[ic][:], lhs,
                                a_t[:, kp, :, ic * 512:(ic + 1) * 512],
                                start=(kp == 0), stop=(kp == NKP - 1),
                                perf_mode=DR)
                    for ic in range(NIC):
                        nc.vector.tensor_mul(
                            pg_sb[:, ic * 512:(ic + 1) * 512], psg[ic][:],
                            st_t[:, ic * 512:(ic + 1) * 512])
                        # chunked so each fires right after its multiply
                        nc.sync.dma_start_transpose(
                            ptT_t[:, ic * 4:(ic + 1) * 4,
                                  jb * 128:(jb + 1) * 128],
                            pg_sb[:, ic * 512:(ic + 1) * 512])
                    # off the critical path: transposes only need pg
                    nc.vector.reduce_sum(
                        rsum_t[:, jb:jb + 1], pg_sb[:],
                        axis=mybir.AxisListType.X)

            # ---- phase X: XT[r, j] = sum_i nf[i, r] * P^T[i, j] ----
            with tc.tile_pool(name="nfp", bufs=1) as nfp, \
                 tc.tile_pool(name="fwp", bufs=1) as fwp, \
                 tc.tile_pool(name="xtp", bufs=1) as xtp:
                nfhi_t = nfp.tile([128, NIB - NLO, F_RAW], BF16)
                nc.sync.dma_start(nfhi_t[:], nf_d[:, NLO:NIB, :])
                nfT_t = nfp.tile([128, NRB, JB], BF16)
                nc.sync.dma_start(nfT_t[:], nfT_d[:])
                fw_t = fwp.tile([128, NRB, C], BF16)
                nc.sync.dma_start(fw_t[:], fw_d[:])
                fw2_t = fwp.tile([128, NRB, C], BF16)
                nc.sync.dma_start(fw2_t[:], fw2_d[:])
                fbt_t = fwp.tile([128, C], BF16)
                nc.sync.dma_start(fbt_t[:], fbt_d[:])
                fb2_t = fwp.tile([128, C], BF16)
                nc.sync.dma_start(fb2_t[:], fb2_d[:])
                xt_sb = xtp.tile([128, NRB, JB], BF16)
                with tc.tile_pool(name="psX", bufs=4,
                                  space=bass.MemorySpace.PSUM) as psxp:
                    psx = [psxp.tile([128, JB], F32, tag="x", name=f"x{i}")
                           for i in range(NRB)]
                    for ib in range(NIB):
                        for rb in range(NRB):
                            rsl = slice(rb * 128, (rb + 1) * 128)
                            if ib < NLO:
                                nfsrc = nflo_t[:, ib, rsl]
                            else:
                                nfsrc = nfhi_t[:, ib - NLO, rsl]
                            nc.tensor.matmul(
                                psx[rb][:], nfsrc, ptT_t[:, ib, :],
                                start=(ib == 0), stop=(ib == NIB - 1))
                    for rb in range(NRB):
                        nc.vector.tensor_copy(xt_sb[:, rb, :], psx[rb][:])

                # ---- phase O: out = (XT.T@FW + rsum*fb) * (nfT.T@FW2 + fb2)
                with tc.tile_pool(name="epp", bufs=2) as epp, \
                     tc.tile_pool(name="psA", bufs=4,
                                  space=bass.MemorySpace.PSUM) as psap, \
                     tc.tile_pool(name="psW", bufs=4,
                                  space=bass.MemorySpace.PSUM) as pswp:
                    for jb in range(NJB):
                        for cq in range(NCQ):
                            psa = [psap.tile([128, 512], F32, tag="pa",
                                             name=f"pa{jb}_{cq}_{i}")
                                   for i in range(2)]
                            psw = [pswp.tile([128, 512], F32, tag="pw",
                                             name=f"pw{jb}_{cq}_{i}")
                                   for i in range(2)]
                            for rb in range(NRB):
                                lhsa = xt_sb[:, rb, jb * 128:(jb + 1) * 128]
                                for cc in range(2):
                                    nc.tensor.matmul(
                                        psa[cc][:], lhsa,
                                        fw_t[:, rb, cq * CQ + cc * 512:
                                             cq * CQ + (cc + 1) * 512],
                                        start=(rb == 0), stop=(rb == NRB - 1))
                                lhsw = nfT_t[:, rb, jb * 128:(jb + 1) * 128]
                                for cc in range(2):
                                    nc.tensor.matmul(
                                        psw[cc][:], lhsw,
                                        fw2_t[:, rb, cq * CQ + cc * 512:
                                              cq * CQ + (cc + 1) * 512],
                                        start=(rb == 0), stop=(rb == NRB - 1))
                            sl = slice(cq * CQ, (cq + 1) * CQ)
                            acc2 = epp.tile([128, CQ], F32, tag="a2",
                                            name=f"a2{jb}_{cq}")
                            nc.scalar.activation(
                                acc2[:], fbt_t[:, sl],
                                mybir.ActivationFunctionType.Identity,
                                bias=0.0, scale=rsum_t[:, jb:jb + 1])
                            wfsc = epp.tile([128, CQ], F32, tag="wsc",
                                            name=f"wsc{jb}_{cq}")
                            t_sb = epp.tile([128, CQ], F32, tag="t",
                                            name=f"t{jb}_{cq}")
                            for cc in range(2):
                                ccs = slice(cc * 512, (cc + 1) * 512)
                                nc.vector.tensor_add(
                                    wfsc[:, ccs], psw[cc][:],
                                    fb2_t[:, cq * CQ + cc * 512:
                                          cq * CQ + (cc + 1) * 512])
                                nc.vector.tensor_add(
                                    t_sb[:, ccs], psa[cc][:], acc2[:, ccs])
                            o_sb = epp.tile([128, CQ], F32, tag="o",
                                            name=f"o{jb}_{cq}")
                            for cc in range(2):
                                ccs = slice(cc * 512, (cc + 1) * 512)
                                nc.vector.tensor_mul(
                                    o_sb[:, ccs], t_sb[:, ccs], wfsc[:, ccs])
                                # scalar DGE ring: stores bypass the input
                                # loads queued on the sync ring
                                nc.scalar.dma_start(
                                    out_d[jb * 128:(jb + 1) * 128,
                                          cq * CQ + cc * 512:
                                          cq * CQ + (cc + 1) * 512],
                                    o_sb[:, ccs])
    nc.finalize()
    return nc


_NC = None


def _get_nc():
    global _NC
    if _NC is None:
        _NC = _build_neff()
    return _NC


def _ensure_trace_hook():
    """Best-effort NTFF profiling shim (test harness only; grading runs
    without tracing)."""
    try:
        from antenv.axon_hooks import get_axon_ntff_profile_hook
        return get_axon_ntff_profile_hook() is not None
    except ImportError:
        pass
    try:
        import types
        if "/root/.axon_site" not in sys.path:
            sys.path.insert(0, "/root/.axon_site")
        from trn_agent_boot.trn_boot import _ntff_profile_via_ctypes
        hook = _ntff_profile_via_ctypes("/opt/axon/libaxon_pjrt.so")
        if hook is None:
            return False
        import antenv
        mod = types.ModuleType("antenv.axon_hooks")
        mod.get_axon_ntff_profile_hook = lambda: hook
        mod.set_axon_ntff_profile_hook = lambda h: None
        sys.modules["antenv.axon_hooks"] = mod
        antenv.axon_hooks = mod
        from concourse import bass_utils as _bu
        _bu.upload_artifacts = lambda tmpdir: ""
        return True
    except Exception:
        return False


def _run(nc, in_maps, cores, trace, tag):
    if trace:
        try:
            r = run_bass_kernel_spmd(nc, in_maps, cores, trace=True)
            LAST_EXEC[tag] = r.exec_time_ns
            LAST_RESULTS[tag] = r
            return r
        except Exception as e:
            print(f"trace run failed ({e!r}); retrying without trace")
    return run_bass_kernel_spmd(nc, in_maps, cores)


def _pack(x, nblk):
    """[nblk*128, F] -> [128, nblk, F] (SBUF layout, row-block p-major)."""
    f = x.shape[1]
    return np.ascontiguousarray(
        x.reshape(nblk, 128, f).transpose(1, 0, 2))


def kernel(node_features, adjacency_matrix, mask_father, neighbor_count,
           mask_hadamard, linear_w, linear_b, weight):
    nc = _get_nc()
    trace = bool(int(os.environ.get("BASS_KERNEL_TRACE", "0"))) and _ensure_trace_hook()
    cores = list(range(M))
    bf = ml_dtypes.bfloat16
    f8 = ml_dtypes.float8_e4m3

    nf = np.asarray(node_features, dtype=np.float32)
    A = np.asarray(adjacency_matrix, dtype=np.float32)
    Ao = np.asarray(mask_father, dtype=np.float32)[:, 0, :]
    S = np.asarray(mask_hadamard, dtype=np.float32)
    ncnt = np.asarray(neighbor_count, dtype=np.float32)
    lw = np.asarray(linear_w, dtype=np.float32)
    lb = np.asarray(linear_b, dtype=np.float32)
    W = np.asarray(weight, dtype=np.float32)

    FW = np.ascontiguousarray(lw.T) @ W                    # [F_RAW, C]
    fb = lb @ W                                            # [C]
    inv2 = (1.0 / np.square(ncnt.astype(np.float64)))[:, 0].astype(np.float32)
    FW2 = FW * inv2[None, :]
    fb2 = fb * inv2

    a_re = _pack(A.astype(f8), NKB)                        # [128, 32, N]
    nf_re = _pack(nf.astype(bf), NIB)                      # [128, 32, F_RAW]
    fw_re = _pack(FW.astype(bf), NRB)                      # [128, 4, C]
    fw2_re = _pack(FW2.astype(bf), NRB)
    fbt = np.ascontiguousarray(np.broadcast_to(fb[None, :], (128, C))).astype(bf)
    fb2t = np.ascontiguousarray(np.broadcast_to(fb2[None, :], (128, C))).astype(bf)

    in_maps = []
    for m in range(M):
        sl = slice(m * JB, (m + 1) * JB)
        in_maps.append({
            "a8": a_re,
            "ao8": _pack(np.ascontiguousarray(Ao[:, sl]).astype(f8), NKB),
            "stT": np.ascontiguousarray(S[:, 0, sl].T).astype(bf),
            "nfb": nf_re,
            "nfT": _pack(np.ascontiguousarray(nf[sl].T).astype(bf), NRB),
            "fwt": fw_re,
            "fw2": fw2_re,
            "fbt": fbt,
            "fb2": fb2t,
        })
    r = _run(nc, in_maps, cores, trace, "neff")

    out = np.empty((C, N), dtype=np.float32)
    for m in range(M):
        out[:, m * JB:(m + 1) * JB] = np.asarray(r.results[m]["outc"]).T
    return out


# revision 33
# speedup vs baseline: 1.0554x; 1.0554x over previous
"""Distributed Bass kernel for nn_Interaction_GraphConvolution.

Math (reference):
    x  = node_features @ linear_w.T + linear_b          [N, IN_F]
    wf = x @ weight                                     [N, C]
    G  = mask_father[:,0,:].T @ adjacency               [N, N]
    P  = G * mask_hadamard[:,0,:].T                     [N, N]
    out[c, j] = wf[j,c] * (P @ wf)[j,c] / neighbor_count[c]^2

Host folds the two linear layers (FW = lw.T @ W, fb = lb @ W) and the
normalization (FW2 = FW / ncnt^2, fb2 = fb / ncnt^2).  The big GEMM is
factored through the linear layer:
    P @ wf = (P @ nf) @ FW + rowsum(P) x fb
which is 2*N^2*F_RAW + 2*N*F_RAW*C flops instead of 2*N^2*C (4x less).

Single SPMD launch, output columns j (node dim) split across 8 cores:
  phase G: G rows J_m via fp8e4 DoubleRow (adjacency/mask are 0/1 -
           exact), multiply by S^T, DMA-XBAR transpose into P^T.
  phase X: XT = nf^T @ P^T (bf16).
  phase O: out[j,c] = (XT.T@FW + rsum*fb) * (nfT.T@FW2 + fb2).
Matmuls keep one 128x128 stationary across multiple 512-wide moving
chunks so LDWEIGHTS amortizes.  Inputs are host-packed into SBUF layout
so every DMA moves >=4KB per partition line (descriptor-rate bound).
"""

import os
import sys

sys.path.insert(0, "/opt/trn_rl_repo")

import numpy as np
import ml_dtypes

from concourse import bass, bacc, mybir, tile
from concourse.bass_utils import run_bass_kernel_spmd

F32 = mybir.dt.float32
BF16 = mybir.dt.bfloat16
F8E4 = mybir.dt.float8e4
DR = mybir.MatmulPerfMode.DoubleRow

N = 4096       # nodes (== out channels C)
F_RAW = 512    # raw feature dim
IN_F = 1024    # hidden dim
C = 4096       # out channels
M = 8          # cores
JB = N // M    # 512 output columns per core

NKP = N // 256    # 16 k-pairs (DoubleRow contracts 256 per pass)
NKB = N // 128    # 32 k-blocks
NIB = N // 128    # 32 i-blocks
NJB = JB // 128   # 4 j-blocks
NIC = N // 512    # 8 i-chunks of 512
NRB = F_RAW // 128  # 4 r-blocks
CQ = 1024         # c quarter
NCQ = C // CQ     # 4

LAST_EXEC = {}
LAST_RESULTS = {}


def _build_neff():
    nc = bacc.Bacc()
    a_d = nc.dram_tensor("a8", [128, NKB, N], F8E4, kind="ExternalInput")
    ao_d = nc.dram_tensor("ao8", [128, NKB, JB], F8E4, kind="ExternalInput")
    st_d = nc.dram_tensor("stT", [JB, N], BF16, kind="ExternalInput")
    nf_d = nc.dram_tensor("nfb", [128, NIB, F_RAW], BF16, kind="ExternalInput")
    nfT_d = nc.dram_tensor("nfT", [128, NRB, JB], BF16, kind="ExternalInput")
    fw_d = nc.dram_tensor("fwt", [128, NRB, C], BF16, kind="ExternalInput")
    fw2_d = nc.dram_tensor("fw2", [128, NRB, C], BF16, kind="ExternalInput")
    fbt_d = nc.dram_tensor("fbt", [128, C], BF16, kind="ExternalInput")
    fb2_d = nc.dram_tensor("fb2", [128, C], BF16, kind="ExternalInput")
    out_d = nc.dram_tensor("outc", [JB, C], F32, kind="ExternalOutput")

    NLO = 12  # nf i-blocks prefetched during phase G (SBUF budget)

    with tile.TileContext(nc) as tc:
        with tc.tile_pool(name="const", bufs=1) as constp:
            # P^T: ptT[p, ib, j] = P[j, ib*128+p], i on partitions
            ptT_t = constp.tile([128, NIB, JB], BF16)
            rsum_t = constp.tile([128, NJB], F32)  # rowsum(P), col per j-block
            nflo_t = constp.tile([128, NLO, F_RAW], BF16)

            # PE warm-up: HAM releases the clock throttle after ~3.4us of
            # activity; burn dummy matmuls while the first DMAs stream.
            with tc.tile_pool(name="wup", bufs=1) as wup, \
                 tc.tile_pool(name="wups", bufs=1,
                              space=bass.MemorySpace.PSUM) as wupp:
                wu = wup.tile([128, 640], BF16)
                nc.gpsimd.memset(wu[:], 0.0)
                wu_ps = [wupp.tile([128, 512], F32, tag="w", name=f"w{i}")
                         for i in range(4)]
                for i in range(12):
                    nc.tensor.matmul(wu_ps[i % 4][:], wu[:, 512:640],
                                     wu[:, 0:512], start=True, stop=True)

            # ---- phase G: G rows J_m (fp8 DoubleRow), *S^T, transpose ----
            with tc.tile_pool(name="ga", bufs=1) as gap, \
                 tc.tile_pool(name="stp", bufs=1) as stp, \
                 tc.tile_pool(name="pgp", bufs=1) as pgp, \
                 tc.tile_pool(name="psG", bufs=8,
                              space=bass.MemorySpace.PSUM) as psgp:
                aot_t = gap.tile([128, NKP, 2, JB], F8E4)
                nc.sync.dma_start(aot_t[:], ao_d[:])
                st0_t = stp.tile([128, N], BF16, tag="st", name="st0")
                nc.sync.dma_start(st0_t[:], st_d[0:128, :])
                a_t = gap.tile([128, NKP, 2, N], F8E4)
                for kp in range(NKP):
                    nc.sync.dma_start(
                        a_t[:, kp, :, :], a_d[:, 2 * kp:2 * kp + 2, :])
                nc.sync.dma_start(nflo_t[:], nf_d[:, 0:NLO, :])

                for jb in range(NJB):
                    if jb == 0:
                        st_t = st0_t
                    else:
                        st_t = stp.tile([128, N], BF16, tag="st",
                                        name=f"st{jb}")
                        nc.sync.dma_start(
                            st_t[:], st_d[jb * 128:(jb + 1) * 128, :])
                    pg_sb = pgp.tile([128, N], BF16, tag="pg", name=f"pg{jb}")
                    psg = [psgp.tile([128, 512], F32, tag="g",
                                     name=f"g{jb}_{i}") for i in range(NIC)]
                    for kp in range(NKP):
                        lhs = aot_t[:, kp, :, jb * 128:(jb + 1) * 128]
                        for ic in range(NIC):
                            nc.tensor.matmul(
                                psg[ic][:], lhs,
                                a_t[:, kp, :, ic * 512:(ic + 1) * 512],
                                start=(kp == 0), stop=(kp == NKP - 1),
                                perf_mode=DR)
                    for ic in range(NIC):
                        nc.vector.tensor_mul(
                            pg_sb[:, ic * 512:(ic + 1) * 512], psg[ic][:],
                            st_t[:, ic * 512:(ic + 1) * 512])
                    nc.sync.dma_start_transpose(
                        ptT_t[:, :, jb * 128:(jb + 1) * 128], pg_sb[:])
                    # off the critical path: transpose above only needs pg
                    nc.vector.reduce_sum(
                        rsum_t[:, jb:jb + 1], pg_sb[:],
                        axis=mybir.AxisListType.X)

            # ---- phase X: XT[r, j] = sum_i nf[i, r] * P^T[i, j] ----
            with tc.tile_pool(name="nfp", bufs=1) as nfp, \
                 tc.tile_pool(name="fwp", bufs=1) as fwp, \
                 tc.tile_pool(name="xtp", bufs=1) as xtp:
                nfhi_t = nfp.tile([128, NIB - NLO, F_RAW], BF16)
                nc.sync.dma_start(nfhi_t[:], nf_d[:, NLO:NIB, :])
                nfT_t = nfp.tile([128, NRB, JB], BF16)
                nc.sync.dma_start(nfT_t[:], nfT_d[:])
                fw_t = fwp.tile([128, NRB, C], BF16)
                nc.sync.dma_start(fw_t[:], fw_d[:])
                fw2_t = fwp.tile([128, NRB, C], BF16)
                nc.sync.dma_start(fw2_t[:], fw2_d[:])
                fbt_t = fwp.tile([128, C], BF16)
                nc.sync.dma_start(fbt_t[:], fbt_d[:])
                fb2_t = fwp.tile([128, C], BF16)
                nc.sync.dma_start(fb2_t[:], fb2_d[:])
                xt_sb = xtp.tile([128, NRB, JB], BF16)
                with tc.tile_pool(name="psX", bufs=4,
                                  space=bass.MemorySpace.PSUM) as psxp:
                    psx = [psxp.tile([128, JB], F32, tag="x", name=f"x{i}")
                           for i in range(NRB)]
                    for ib in range(NIB):
                        for rb in range(NRB):
                            rsl = slice(rb * 128, (rb + 1) * 128)
                            if ib < NLO:
                                nfsrc = nflo_t[:, ib, rsl]
                            else:
                                nfsrc = nfhi_t[:, ib - NLO, rsl]
                            nc.tensor.matmul(
                                psx[rb][:], nfsrc, ptT_t[:, ib, :],
                                start=(ib == 0), stop=(ib == NIB - 1))
                    for rb in range(NRB):
                        nc.vector.tensor_copy(xt_sb[:, rb, :], psx[rb][:])

                # ---- phase O: out = (XT.T@FW + rsum*fb) * (nfT.T@FW2 + fb2)
                with tc.tile_pool(name="epp", bufs=2) as epp, \
                     tc.tile_pool(name="psA", bufs=4,
                                  space=bass.MemorySpace.PSUM) as psap, \
                     tc.tile_pool(name="psW", bufs=4,
                                  space=bass.MemorySpace.PSUM) as pswp:
                    for jb in range(NJB):
                        for cq in range(NCQ):
                            psa = [psap.tile([128, 512], F32, tag="pa",
                                             name=f"pa{jb}_{cq}_{i}")
                                   for i in range(2)]
                            psw = [pswp.tile([128, 512], F32, tag="pw",
                                             name=f"pw{jb}_{cq}_{i}")
                                   for i in range(2)]
                            for rb in range(NRB):
                                lhsa = xt_sb[:, rb, jb * 128:(jb + 1) * 128]
                                for cc in range(2):
                                    nc.tensor.matmul(
                                        psa[cc][:], lhsa,
                                        fw_t[:, rb, cq * CQ + cc * 512:
                                             cq * CQ + (cc + 1) * 512],
                                        start=(rb == 0), stop=(rb == NRB - 1))
                                lhsw = nfT_t[:, rb, jb * 128:(jb + 1) * 128]
                                for cc in range(2):
                                    nc.tensor.matmul(
                                        psw[cc][:], lhsw,
                                        fw2_t[:, rb, cq * CQ + cc * 512:
                                              cq * CQ + (cc + 1) * 512],
                                        start=(rb == 0), stop=(rb == NRB - 1))
                            sl = slice(cq * CQ, (cq + 1) * CQ)
                            acc2 = epp.tile([128, CQ], F32, tag="a2",
                                            name=f"a2{jb}_{cq}")
                            nc.scalar.activation(
                                acc2[:], fbt_t[:, sl],
                                mybir.ActivationFunctionType.Identity,
                                bias=0.0, scale=rsum_t[:, jb:jb + 1])
                            wfsc = epp.tile([128, CQ], F32, tag="wsc",
                                            name=f"wsc{jb}_{cq}")
                            t_sb = epp.tile([128, CQ], F32, tag="t",
                                            name=f"t{jb}_{cq}")
                            for cc in range(2):
                                ccs = slice(cc * 512, (cc + 1) * 512)
                                nc.vector.tensor_add(
                                    wfsc[:, ccs], psw[cc][:],
                                    fb2_t[:, cq * CQ + cc * 512:
                                          cq * CQ + (cc + 1) * 512])
                                nc.vector.tensor_add(
                                    t_sb[:, ccs], psa[cc][:], acc2[:, ccs])
                            o_sb = epp.tile([128, CQ], F32, tag="o",
                                            name=f"o{jb}_{cq}")
                            for cc in range(2):
                                ccs = slice(cc * 512, (cc + 1) * 512)
                                nc.vector.tensor_mul(
                                    o_sb[:, ccs], t_sb[:, ccs], wfsc[:, ccs])
                                # scalar DGE ring: stores bypass the input
                                # loads queued on the sync ring
                                nc.scalar.dma_start(
                                    out_d[jb * 128:(jb + 1) * 128,
                                          cq * CQ + cc * 512:
                                          cq * CQ + (cc + 1) * 512],
                                    o_sb[:, ccs])
    nc.finalize()
    return nc


_NC = None


def _get_nc():
    global _NC
    if _NC is None:
        _NC = _build_neff()
    return _NC


def _ensure_trace_hook():
    """Best-effort NTFF profiling shim (test harness only; grading runs
    without tracing)."""
    try:
        from antenv.axon_hooks import get_axon_ntff_profile_hook
        return get_axon_ntff_profile_hook() is not None
    except ImportError:
        pass
    try:
        import types
        if "/root/.axon_site" not in sys.path:
            sys.path.insert(0, "/root/.axon_site")
        from trn_agent_boot.trn_boot import _ntff_profile_via_ctypes
        hook = _ntff_profile_via_ctypes("/opt/axon/libaxon_pjrt.so")
        if hook is None:
            return False
        import antenv
        mod = types.ModuleType("antenv.axon_hooks")
        mod.get_axon_ntff_profile_hook = lambda: hook
        mod.set_axon_ntff_profile_hook = lambda h: None
        sys.modules["antenv.axon_hooks"] = mod
        antenv.axon_hooks = mod
        from concourse import bass_utils as _bu
        _bu.upload_artifacts = lambda tmpdir: ""
        return True
    except Exception:
        return False


def _run(nc, in_maps, cores, trace, tag):
    if trace:
        try:
            r = run_bass_kernel_spmd(nc, in_maps, cores, trace=True)
            LAST_EXEC[tag] = r.exec_time_ns
            LAST_RESULTS[tag] = r
            return r
        except Exception as e:
            print(f"trace run failed ({e!r}); retrying without trace")
    return run_bass_kernel_spmd(nc, in_maps, cores)


def _pack(x, nblk):
    """[nblk*128, F] -> [128, nblk, F] (SBUF layout, row-block p-major)."""
    f = x.shape[1]
    return np.ascontiguousarray(
        x.reshape(nblk, 128, f).transpose(1, 0, 2))


def kernel(node_features, adjacency_matrix, mask_father, neighbor_count,
           mask_hadamard, linear_w, linear_b, weight):
    nc = _get_nc()
    trace = bool(int(os.environ.get("BASS_KERNEL_TRACE", "0"))) and _ensure_trace_hook()
    cores = list(range(M))
    bf = ml_dtypes.bfloat16
    f8 = ml_dtypes.float8_e4m3

    nf = np.asarray(node_features, dtype=np.float32)
    A = np.asarray(adjacency_matrix, dtype=np.float32)
    Ao = np.asarray(mask_father, dtype=np.float32)[:, 0, :]
    S = np.asarray(mask_hadamard, dtype=np.float32)
    ncnt = np.asarray(neighbor_count, dtype=np.float32)
    lw = np.asarray(linear_w, dtype=np.float32)
    lb = np.asarray(linear_b, dtype=np.float32)
    W = np.asarray(weight, dtype=np.float32)

    FW = np.ascontiguousarray(lw.T) @ W                    # [F_RAW, C]
    fb = lb @ W                                            # [C]
    inv2 = (1.0 / np.square(ncnt.astype(np.float64)))[:, 0].astype(np.float32)
    FW2 = FW * inv2[None, :]
    fb2 = fb * inv2

    a_re = _pack(A.astype(f8), NKB)                        # [128, 32, N]
    nf_re = _pack(nf.astype(bf), NIB)                      # [128, 32, F_RAW]
    fw_re = _pack(FW.astype(bf), NRB)                      # [128, 4, C]
    fw2_re = _pack(FW2.astype(bf), NRB)
    fbt = np.ascontiguousarray(np.broadcast_to(fb[None, :], (128, C))).astype(bf)
    fb2t = np.ascontiguousarray(np.broadcast_to(fb2[None, :], (128, C))).astype(bf)

    in_maps = []
    for m in range(M):
        sl = slice(m * JB, (m + 1) * JB)
        in_maps.append({
            "a8": a_re,
            "ao8": _pack(np.ascontiguousarray(Ao[:, sl]).astype(f8), NKB),
            "stT": np.ascontiguousarray(S[:, 0, sl].T).astype(bf),
            "nfb": nf_re,
            "nfT": _pack(np.ascontiguousarray(nf[sl].T).astype(bf), NRB),
            "fwt": fw_re,
            "fw2": fw2_re,
            "fbt": fbt,
            "fb2": fb2t,
        })
    r = _run(nc, in_maps, cores, trace, "neff")

    out = np.empty((C, N), dtype=np.float32)
    for m in range(M):
        out[:, m * JB:(m + 1) * JB] = np.asarray(r.results[m]["outc"]).T
    return out


# revision 34
# speedup vs baseline: 1.0665x; 1.0105x over previous
"""Distributed Bass kernel for nn_Interaction_GraphConvolution.

Math (reference):
    x  = node_features @ linear_w.T + linear_b          [N, IN_F]
    wf = x @ weight                                     [N, C]
    G  = mask_father[:,0,:].T @ adjacency               [N, N]
    P  = G * mask_hadamard[:,0,:].T                     [N, N]
    out[c, j] = wf[j,c] * (P @ wf)[j,c] / neighbor_count[c]^2

Host folds the two linear layers (FW = lw.T @ W, fb = lb @ W) and the
normalization (FW2 = FW / ncnt^2, fb2 = fb / ncnt^2).  The big GEMM is
factored through the linear layer:
    P @ wf = (P @ nf) @ FW + rowsum(P) x fb
which is 2*N^2*F_RAW + 2*N*F_RAW*C flops instead of 2*N^2*C (4x less).

Single SPMD launch, output columns j (node dim) split across 8 cores:
  phase G: G rows J_m via fp8e4 DoubleRow (adjacency/mask are 0/1 -
           exact), multiply by S^T, DMA-XBAR transpose into P^T.
  phase X: XT = nf^T @ P^T (bf16).
  phase O: out[j,c] = (XT.T@FW + rsum*fb) * (nfT.T@FW2 + fb2).
Matmuls keep one 128x128 stationary across multiple 512-wide moving
chunks so LDWEIGHTS amortizes.  Inputs are host-packed into SBUF layout
so every DMA moves >=4KB per partition line (descriptor-rate bound).
"""

import os
import sys

sys.path.insert(0, "/opt/trn_rl_repo")

import numpy as np
import ml_dtypes

from concourse import bass, bacc, mybir, tile
from concourse.bass_utils import run_bass_kernel_spmd

F32 = mybir.dt.float32
BF16 = mybir.dt.bfloat16
F8E4 = mybir.dt.float8e4
DR = mybir.MatmulPerfMode.DoubleRow

N = 4096       # nodes (== out channels C)
F_RAW = 512    # raw feature dim
IN_F = 1024    # hidden dim
C = 4096       # out channels
M = 8          # cores
JB = N // M    # 512 output columns per core

NKP = N // 256    # 16 k-pairs (DoubleRow contracts 256 per pass)
NKB = N // 128    # 32 k-blocks
NIB = N // 128    # 32 i-blocks
NJB = JB // 128   # 4 j-blocks
NIC = N // 512    # 8 i-chunks of 512
NRB = F_RAW // 128  # 4 r-blocks
CQ = 1024         # c quarter
NCQ = C // CQ     # 4

LAST_EXEC = {}
LAST_RESULTS = {}


def _build_neff():
    nc = bacc.Bacc()
    a_d = nc.dram_tensor("a8", [128, NKB, N], F8E4, kind="ExternalInput")
    ao_d = nc.dram_tensor("ao8", [128, NKB, JB], F8E4, kind="ExternalInput")
    st_d = nc.dram_tensor("stT", [JB, N], BF16, kind="ExternalInput")
    nf_d = nc.dram_tensor("nfb", [128, NIB, F_RAW], BF16, kind="ExternalInput")
    nfT_d = nc.dram_tensor("nfT", [128, NRB, JB], BF16, kind="ExternalInput")
    fw_d = nc.dram_tensor("fwt", [128, NRB, C], BF16, kind="ExternalInput")
    fw2_d = nc.dram_tensor("fw2", [128, NRB, C], BF16, kind="ExternalInput")
    fbt_d = nc.dram_tensor("fbt", [128, C], BF16, kind="ExternalInput")
    fb2_d = nc.dram_tensor("fb2", [128, C], BF16, kind="ExternalInput")
    out_d = nc.dram_tensor("outc", [JB, C], F32, kind="ExternalOutput")

    NLO = 12  # nf i-blocks prefetched during phase G (SBUF budget)

    with tile.TileContext(nc) as tc:
        with tc.tile_pool(name="const", bufs=1) as constp:
            # P^T: ptT[p, ib, j] = P[j, ib*128+p], i on partitions
            ptT_t = constp.tile([128, NIB, JB], BF16)
            rsum_t = constp.tile([128, NJB], F32)  # rowsum(P), col per j-block
            nflo_t = constp.tile([128, NLO, F_RAW], BF16)

            # PE warm-up: HAM releases the clock throttle after ~3.4us of
            # activity; burn dummy matmuls while the first DMAs stream.
            with tc.tile_pool(name="wup", bufs=1) as wup, \
                 tc.tile_pool(name="wups", bufs=1,
                              space=bass.MemorySpace.PSUM) as wupp:
                wu = wup.tile([128, 640], BF16)
                nc.gpsimd.memset(wu[:], 0.0)
                wu_ps = [wupp.tile([128, 512], F32, tag="w", name=f"w{i}")
                         for i in range(4)]
                for i in range(12):
                    nc.tensor.matmul(wu_ps[i % 4][:], wu[:, 512:640],
                                     wu[:, 0:512], start=True, stop=True)

            # ---- phase G: G rows J_m (fp8 DoubleRow), *S^T, transpose ----
            with tc.tile_pool(name="ga", bufs=1) as gap, \
                 tc.tile_pool(name="stp", bufs=1) as stp, \
                 tc.tile_pool(name="pgp", bufs=1) as pgp, \
                 tc.tile_pool(name="psG", bufs=8,
                              space=bass.MemorySpace.PSUM) as psgp:
                aot_t = gap.tile([128, NKP, 2, JB], F8E4)
                nc.sync.dma_start(aot_t[:], ao_d[:])
                st0_t = stp.tile([128, N], BF16, tag="st", name="st0")
                nc.sync.dma_start(st0_t[:], st_d[0:128, :])
                a_t = gap.tile([128, NKP, 2, N], F8E4)
                for kp in range(NKP):
                    nc.sync.dma_start(
                        a_t[:, kp, :, :], a_d[:, 2 * kp:2 * kp + 2, :])
                nc.sync.dma_start(nflo_t[:], nf_d[:, 0:NLO, :])

                for jb in range(NJB):
                    if jb == 0:
                        st_t = st0_t
                    else:
                        st_t = stp.tile([128, N], BF16, tag="st",
                                        name=f"st{jb}")
                        nc.sync.dma_start(
                            st_t[:], st_d[jb * 128:(jb + 1) * 128, :])
                    pg_sb = pgp.tile([128, N], BF16, tag="pg", name=f"pg{jb}")
                    psg = [psgp.tile([128, 512], F32, tag="g",
                                     name=f"g{jb}_{i}") for i in range(NIC)]
                    for kp in range(NKP):
                        lhs = aot_t[:, kp, :, jb * 128:(jb + 1) * 128]
                        for ic in range(NIC):
                            nc.tensor.matmul(
                                psg[ic][:], lhs,
                                a_t[:, kp, :, ic * 512:(ic + 1) * 512],
                                start=(kp == 0), stop=(kp == NKP - 1),
                                perf_mode=DR)
                    for ic in range(NIC):
                        nc.vector.tensor_mul(
                            pg_sb[:, ic * 512:(ic + 1) * 512], psg[ic][:],
                            st_t[:, ic * 512:(ic + 1) * 512])
                    nc.sync.dma_start_transpose(
                        ptT_t[:, :, jb * 128:(jb + 1) * 128], pg_sb[:])
                    # off the critical path: transpose above only needs pg
                    nc.vector.reduce_sum(
                        rsum_t[:, jb:jb + 1], pg_sb[:],
                        axis=mybir.AxisListType.X)

            # ---- phase X: XT[r, j] = sum_i nf[i, r] * P^T[i, j] ----
            with tc.tile_pool(name="nfp", bufs=1) as nfp, \
                 tc.tile_pool(name="fwp", bufs=1) as fwp, \
                 tc.tile_pool(name="xtp", bufs=1) as xtp:
                nfhi_t = nfp.tile([128, NIB - NLO, F_RAW], BF16)
                nfT_t = nfp.tile([128, NRB, JB], BF16)
                fw_t = fwp.tile([128, NRB, C], BF16)
                fw2_t = fwp.tile([128, NRB, C], BF16)
                fbt_t = fwp.tile([128, C], BF16)
                fb2_t = fwp.tile([128, C], BF16)
                xt_sb = xtp.tile([128, NRB, JB], BF16)
                with tc.tile_pool(name="psX", bufs=4,
                                  space=bass.MemorySpace.PSUM) as psxp:
                    psx = [psxp.tile([128, JB], F32, tag="x", name=f"x{i}")
                           for i in range(NRB)]
                    for ib in range(NIB):
                        # issue the big phase-X/O loads only after the first
                        # matmuls, so the boundary MMs wait on nflo/ptT alone
                        if ib == 1:
                            nc.sync.dma_start(nfhi_t[:], nf_d[:, NLO:NIB, :])
                        elif ib == 3:
                            nc.sync.dma_start(fw_t[:], fw_d[:])
                        elif ib == 5:
                            nc.sync.dma_start(nfT_t[:], nfT_d[:])
                            nc.sync.dma_start(fw2_t[:], fw2_d[:])
                        elif ib == 7:
                            nc.sync.dma_start(fbt_t[:], fbt_d[:])
                            nc.sync.dma_start(fb2_t[:], fb2_d[:])
                        for rb in range(NRB):
                            rsl = slice(rb * 128, (rb + 1) * 128)
                            if ib < NLO:
                                nfsrc = nflo_t[:, ib, rsl]
                            else:
                                nfsrc = nfhi_t[:, ib - NLO, rsl]
                            nc.tensor.matmul(
                                psx[rb][:], nfsrc, ptT_t[:, ib, :],
                                start=(ib == 0), stop=(ib == NIB - 1))
                    for rb in range(NRB):
                        nc.vector.tensor_copy(xt_sb[:, rb, :], psx[rb][:])

                # ---- phase O: out = (XT.T@FW + rsum*fb) * (nfT.T@FW2 + fb2)
                with tc.tile_pool(name="epp", bufs=2) as epp, \
                     tc.tile_pool(name="psA", bufs=4,
                                  space=bass.MemorySpace.PSUM) as psap, \
                     tc.tile_pool(name="psW", bufs=4,
                                  space=bass.MemorySpace.PSUM) as pswp:
                    for jb in range(NJB):
                        for cq in range(NCQ):
                            psa = [psap.tile([128, 512], F32, tag="pa",
                                             name=f"pa{jb}_{cq}_{i}")
                                   for i in range(2)]
                            psw = [pswp.tile([128, 512], F32, tag="pw",
                                             name=f"pw{jb}_{cq}_{i}")
                                   for i in range(2)]
                            for rb in range(NRB):
                                lhsa = xt_sb[:, rb, jb * 128:(jb + 1) * 128]
                                for cc in range(2):
                                    nc.tensor.matmul(
                                        psa[cc][:], lhsa,
                                        fw_t[:, rb, cq * CQ + cc * 512:
                                             cq * CQ + (cc + 1) * 512],
                                        start=(rb == 0), stop=(rb == NRB - 1))
                                lhsw = nfT_t[:, rb, jb * 128:(jb + 1) * 128]
                                for cc in range(2):
                                    nc.tensor.matmul(
                                        psw[cc][:], lhsw,
                                        fw2_t[:, rb, cq * CQ + cc * 512:
                                              cq * CQ + (cc + 1) * 512],
                                        start=(rb == 0), stop=(rb == NRB - 1))
                            sl = slice(cq * CQ, (cq + 1) * CQ)
                            acc2 = epp.tile([128, CQ], F32, tag="a2",
                                            name=f"a2{jb}_{cq}")
                            nc.scalar.activation(
                                acc2[:], fbt_t[:, sl],
                                mybir.ActivationFunctionType.Identity,
                                bias=0.0, scale=rsum_t[:, jb:jb + 1])
                            wfsc = epp.tile([128, CQ], F32, tag="wsc",
                                            name=f"wsc{jb}_{cq}")
                            t_sb = epp.tile([128, CQ], F32, tag="t",
                                            name=f"t{jb}_{cq}")
                            for cc in range(2):
                                ccs = slice(cc * 512, (cc + 1) * 512)
                                nc.vector.tensor_add(
                                    wfsc[:, ccs], psw[cc][:],
                                    fb2_t[:, cq * CQ + cc * 512:
                                          cq * CQ + (cc + 1) * 512])
                                nc.vector.tensor_add(
                                    t_sb[:, ccs], psa[cc][:], acc2[:, ccs])
                            o_sb = epp.tile([128, CQ], F32, tag="o",
                                            name=f"o{jb}_{cq}")
                            for cc in range(2):
                                ccs = slice(cc * 512, (cc + 1) * 512)
                                nc.vector.tensor_mul(
                                    o_sb[:, ccs], t_sb[:, ccs], wfsc[:, ccs])
                                # scalar DGE ring: stores bypass the input
                                # loads queued on the sync ring
                                nc.scalar.dma_start(
                                    out_d[jb * 128:(jb + 1) * 128,
                                          cq * CQ + cc * 512:
                                          cq * CQ + (cc + 1) * 512],
                                    o_sb[:, ccs])
    nc.finalize()
    return nc


_NC = None


def _get_nc():
    global _NC
    if _NC is None:
        _NC = _build_neff()
    return _NC


def _ensure_trace_hook():
    """Best-effort NTFF profiling shim (test harness only; grading runs
    without tracing)."""
    try:
        from antenv.axon_hooks import get_axon_ntff_profile_hook
        return get_axon_ntff_profile_hook() is not None
    except ImportError:
        pass
    try:
        import types
        if "/root/.axon_site" not in sys.path:
            sys.path.insert(0, "/root/.axon_site")
        from trn_agent_boot.trn_boot import _ntff_profile_via_ctypes
        hook = _ntff_profile_via_ctypes("/opt/axon/libaxon_pjrt.so")
        if hook is None:
            return False
        import antenv
        mod = types.ModuleType("antenv.axon_hooks")
        mod.get_axon_ntff_profile_hook = lambda: hook
        mod.set_axon_ntff_profile_hook = lambda h: None
        sys.modules["antenv.axon_hooks"] = mod
        antenv.axon_hooks = mod
        from concourse import bass_utils as _bu
        _bu.upload_artifacts = lambda tmpdir: ""
        return True
    except Exception:
        return False


def _run(nc, in_maps, cores, trace, tag):
    if trace:
        try:
            r = run_bass_kernel_spmd(nc, in_maps, cores, trace=True)
            LAST_EXEC[tag] = r.exec_time_ns
            LAST_RESULTS[tag] = r
            return r
        except Exception as e:
            print(f"trace run failed ({e!r}); retrying without trace")
    return run_bass_kernel_spmd(nc, in_maps, cores)


def _pack(x, nblk):
    """[nblk*128, F] -> [128, nblk, F] (SBUF layout, row-block p-major)."""
    f = x.shape[1]
    return np.ascontiguousarray(
        x.reshape(nblk, 128, f).transpose(1, 0, 2))


def kernel(node_features, adjacency_matrix, mask_father, neighbor_count,
           mask_hadamard, linear_w, linear_b, weight):
    nc = _get_nc()
    trace = bool(int(os.environ.get("BASS_KERNEL_TRACE", "0"))) and _ensure_trace_hook()
    cores = list(range(M))
    bf = ml_dtypes.bfloat16
    f8 = ml_dtypes.float8_e4m3

    nf = np.asarray(node_features, dtype=np.float32)
    A = np.asarray(adjacency_matrix, dtype=np.float32)
    Ao = np.asarray(mask_father, dtype=np.float32)[:, 0, :]
    S = np.asarray(mask_hadamard, dtype=np.float32)
    ncnt = np.asarray(neighbor_count, dtype=np.float32)
    lw = np.asarray(linear_w, dtype=np.float32)
    lb = np.asarray(linear_b, dtype=np.float32)
    W = np.asarray(weight, dtype=np.float32)

    FW = np.ascontiguousarray(lw.T) @ W                    # [F_RAW, C]
    fb = lb @ W                                            # [C]
    inv2 = (1.0 / np.square(ncnt.astype(np.float64)))[:, 0].astype(np.float32)
    FW2 = FW * inv2[None, :]
    fb2 = fb * inv2

    a_re = _pack(A.astype(f8), NKB)                        # [128, 32, N]
    nf_re = _pack(nf.astype(bf), NIB)                      # [128, 32, F_RAW]
    fw_re = _pack(FW.astype(bf), NRB)                      # [128, 4, C]
    fw2_re = _pack(FW2.astype(bf), NRB)
    fbt = np.ascontiguousarray(np.broadcast_to(fb[None, :], (128, C))).astype(bf)
    fb2t = np.ascontiguousarray(np.broadcast_to(fb2[None, :], (128, C))).astype(bf)

    in_maps = []
    for m in range(M):
        sl = slice(m * JB, (m + 1) * JB)
        in_maps.append({
            "a8": a_re,
            "ao8": _pack(np.ascontiguousarray(Ao[:, sl]).astype(f8), NKB),
            "stT": np.ascontiguousarray(S[:, 0, sl].T).astype(bf),
            "nfb": nf_re,
            "nfT": _pack(np.ascontiguousarray(nf[sl].T).astype(bf), NRB),
            "fwt": fw_re,
            "fw2": fw2_re,
            "fbt": fbt,
            "fb2": fb2t,
        })
    r = _run(nc, in_maps, cores, trace, "neff")

    out = np.empty((C, N), dtype=np.float32)
    for m in range(M):
        out[:, m * JB:(m + 1) * JB] = np.asarray(r.results[m]["outc"]).T
    return out
